# revision 48
# baseline (speedup 1.0000x reference)
"""Trainium2 Bass kernel for nn_CWDiscriminator (per-class 3-layer MLP).

reference:
    x = inputs.transpose(0, 2, 1)            # (B, C, F)
    h = relu(einsum('bcf,cfg->bcg', x, W1) + b1)
    h = relu(einsum('bcf,cfg->bcg', h, W2) + b2)
    out = einsum('bcf,cf->bc', h, W3) + b3   # (B, C)

B=16384, F=256, C=19. Data-parallel over B across 8 NeuronCores
(B_loc = 2048 per core). CLASS-MAJOR schedule: outer loop over classes;
within a slot, G1(c) and G2(c-1) are interleaved at SECTION level
(four 512-column batch sections), so ACT (h1 eviction) and DVE (h2
eviction) load alternates 1:1 and the PE never outruns the 7 rotating
PSUM banks. Class 0 chains G1(0)/G2(0) in its own slot (halves the
startup data-demand rate while the DMA queue fills).

Per class c, per section h:
  - GEMM1 (bf16): H1.T = W1[c].T @ X.T -> PSUM, ACT evicts fused
    bias+ReLU to bf16.
  - GEMM2 (bf16): H2.T = W2[c].T @ H1.T -> PSUM, DVE evicts fused
    bias+ReLU to bf16 (ACT for the last class).
  - GEMM3: col-tiled, batched per group of 4 classes (r = c//4,
    strip j = c%4). The four classes' matmuls go to the four
    32-partition column groups via tile_position=(0,32j) and run
    concurrently, all accumulating into ONE shared PSUM bank at
    partition 32j + 5h + r (5h+r is a bijection onto 0..19, so each
    strip's 20 rows are dense); section h is selected by leading-zero
    columns in the lhsT. A zero matmul at kernel start sets
    has_written for the bank (doubling as PE warm-up), so strip
    matmuls never need start=True (whose bank-wide clear would race
    between concurrent strips). The final burst is interleaved into
    the last slot, one section lagged, to shorten the kernel tail.

Schedule lore (measured on trn2, see _transcript):
  - The PE clock ramps 0.65 -> 1.2 -> 2.4 GHz over ~3 us of
    UNINTERRUPTED execution, and any idle gap resets it; dep-free
    narrow warm-up/filler matmuls bridge every startup supply stall.
  - DMA completion semaphores appear to fire per DGE batch, ~0.9 us
    after transfer end; first readers of a DMA pay a ~165 ns
    pipeline-break for the sem check even when long satisfied.
  - X streams on the sync queue (kept pure at the start: readers can
    be gated on LATER same-queue DMAs); class-0 weights ride the
    scalar queue so both DGE pipe-fill latencies overlap. gpsimd's
    queue is unusable early (its post-DMA DRAIN delays the sem ~3 us).
Output per core: one padded DMA to dram [4,32,SEC]; host takes rows
:20 per strip, reshapes, transposes, adds b3.
"""

import sys
import types

import numpy as np
import ml_dtypes

B, F, C = 16384, 256, 19
NCORES = 8
B_LOC = B // NCORES          # 2048
NSEC = 4
SEC = B_LOC // NSEC          # 512
BF16 = ml_dtypes.bfloat16

# strip j = c % 4, row r = c // 4; classes per strip column
STRIP_N = [5, 5, 5, 4]


# ---------------------------------------------------------------------------
# axon environment shims (NTFF profile hook + artifact upload stub) and the
# one-wait-per-instruction legalizer this walrus build requires.
# ---------------------------------------------------------------------------

def _setup_axon_env():
    if 'antenv.axon_hooks' not in sys.modules:
        mod = types.ModuleType('antenv.axon_hooks')
        mod._hook = None
        mod.set_axon_ntff_profile_hook = lambda h: setattr(mod, '_hook', h)
        mod.get_axon_ntff_profile_hook = lambda: mod._hook
        sys.modules['antenv.axon_hooks'] = mod
        try:
            import antenv
            antenv.axon_hooks = mod
        except ImportError:
            pass
        try:
            from trn_agent_boot.trn_boot import _ntff_profile_via_ctypes
            mod._hook = _ntff_profile_via_ctypes('/opt/axon/libaxon_pjrt.so')
        except Exception:
            pass
    import concourse.bass_utils as bu
    bu.upload_artifacts = lambda tmpdir: 'file://' + str(tmpdir)


def _legalize_waits(nc):
    """walrus accepts at most ONE sync wait per engine instruction (2 for
    EventSemaphore). Split extras onto preceding same-engine NoOps."""
    import concourse.mybir as mybir
    n_split = 0
    for fn in nc.m.functions:
        for bb in fn.blocks:
            insts = bb.instructions
            out = []
            for inst in insts:
                si = inst.sync_info
                ow = list(si.on_wait) if si is not None and si.on_wait else []
                cap = 2 if inst.opcode == "EventSemaphore" else 1
                if len(ow) > cap:
                    keep = ow[-cap:]
                    for k, w in enumerate(ow[:-cap]):
                        nop = mybir.InstNoOp(
                            name=f"{inst.name}-wsplit{k}",
                            engine=inst.engine,
                            ins=[],
                            outs=[],
                            sync_info=mybir.SyncInfo(on_wait=[w], on_update=[]),
                        )
                        out.append(nop)
                        n_split += 1
                    inst.sync_info = mybir.SyncInfo(
                        on_wait=keep,
                        on_update=list(si.on_update) if si.on_update else [],
                    )
                out.append(inst)
            insts[:] = out
    return n_split


# ---------------------------------------------------------------------------
# device program
# ---------------------------------------------------------------------------

_CACHE = {}
last_results = None  # BassKernelResults of the most recent run (for test.py)

# class-range chunks for weight loads (finer first so class 0 starts
# early; coarse later — fewer DMAs means fewer semaphore-gated readers)
W_CHUNKS = [(0, 1), (1, 2), (2, 4), (4, 8), (8, 13), (13, C)]


def _build_program():
    from contextlib import ExitStack
    import concourse.bass as bass
    import concourse.mybir as mybir
    import concourse.tile as tile

    F32 = mybir.dt.float32
    B16 = mybir.dt.bfloat16

    nc = bass.Bass()

    # xt01[c, p, k, b] = x[b, 128k+p, c] for classes 0,1 (fine-grained
    # startup); xtp[pr, p, k, j, b] packs class pairs (2+2pr, 3+2pr) so
    # one DMA (and one semaphore-gated first reader) covers two classes;
    # xt18 is the odd class out.
    xt01 = nc.declare_dram_parameter("xt01", [2, 128, 2, B_LOC], B16,
                                     isOutput=False)
    xtp = nc.declare_dram_parameter("xtp", [8, 128, 2, 2, B_LOC], B16,
                                    isOutput=False)
    xt18 = nc.declare_dram_parameter("xt18", [128, 2, B_LOC], B16,
                                     isOutput=False)
    w1t = nc.declare_dram_parameter("w1t", [128, C, 2, 2, 128], B16,
                                    isOutput=False)
    w2t = nc.declare_dram_parameter("w2t", [128, C, 2, 2, 128], B16,
                                    isOutput=False)
    # w3z[p, c, k, h, i] = (i == 8h + c//4) * W3[c, 128k+p]
    w3z = nc.declare_dram_parameter("w3z", [128, C, 2, NSEC, 20], B16,
                                    isOutput=False)
    b1s = nc.declare_dram_parameter("b1s", [128, C, 2], F32, isOutput=False)
    b2s = nc.declare_dram_parameter("b2s", [128, C, 2], F32, isOutput=False)
    # out[j, q, b]: q = 5h+r -> class 4r+j, batch column 512h+b
    # (rows 20..31 per strip are padding so one 128-partition DMA works)
    out = nc.declare_dram_parameter("out", [4, 32, SEC], F32,
                                    isOutput=True)

    with ExitStack() as ctx:
        tc = ctx.enter_context(tile.TileContext(nc))

        consts = ctx.enter_context(tc.tile_pool(name="consts", bufs=1))
        xt_pool = ctx.enter_context(tc.tile_pool(name="xt", bufs=2))
        xtp_pool = ctx.enter_context(tc.tile_pool(name="xtp", bufs=3))
        h1_pool = ctx.enter_context(tc.tile_pool(name="h1p", bufs=8))
        h2_pool = ctx.enter_context(tc.tile_pool(name="h2p", bufs=24))
        out_pool = ctx.enter_context(tc.tile_pool(name="outp", bufs=1))

        ps_g = ctx.enter_context(
            tc.tile_pool(name="ps_g", bufs=7, space="PSUM"))
        ps_3 = ctx.enter_context(
            tc.tile_pool(name="ps_3", bufs=1, space="PSUM"))

        # ---- SBUF tiles for weights/biases
        w1sb = consts.tile([128, C, 2, 2, 128], B16)
        w2sb = consts.tile([128, C, 2, 2, 128], B16)
        w3sb = consts.tile([128, C, 2, NSEC, 20], B16)
        b1sb = consts.tile([128, C, 2], F32)
        b2sb = consts.tile([128, C, 2], F32)

        # ---- DMA: sync queue = pure X stream at the start (readers can
        # end up gated on LATER same-queue DMAs, so nothing may delay the
        # first sections); class-0 weights + biases go on the scalar
        # queue, whose DGE pipeline-fill latency overlaps sync's. gpsimd
        # is NOT used: its post-DMA DRAIN delays the semaphore ~3 us.
        # w3z deferred (first needed at the class-8 burst, ~40 us in).
        xts01 = [xt_pool.tile([128, 2, B_LOC], B16, tag="xt",
                              name=f"xt{c}") for c in range(2)]
        xtps = [xtp_pool.tile([128, 2, 2, B_LOC], B16, tag="xtp",
                              name=f"xtp{p}") for p in range(8)]
        xts18 = xt_pool.tile([128, 2, B_LOC], B16, tag="xt", name="xt18")

        def x_ap(c, k, lo, hi):
            if c < 2:
                return xts01[c][:, k, lo:hi]
            if c == 18:
                return xts18[:, k, lo:hi]
            return xtps[(c - 2) // 2][:, k, (c - 2) % 2, lo:hi]

        for h in range(NSEC):
            nc.sync.dma_start(xts01[0][:, :, h * SEC:(h + 1) * SEC],
                              xt01[0, :, :, h * SEC:(h + 1) * SEC])
        nc.scalar.dma_start(w1sb[:, 0:1], w1t[:, 0:1])
        nc.scalar.dma_start(b1sb[:], b1s[:])
        nc.scalar.dma_start(w2sb[:, 0:1], w2t[:, 0:1])
        nc.scalar.dma_start(b2sb[:], b2s[:])
        nc.sync.dma_start(xts01[1][:, :, 0:B_LOC // 2],
                          xt01[1, :, :, 0:B_LOC // 2])
        nc.sync.dma_start(xts01[1][:, :, B_LOC // 2:],
                          xt01[1, :, :, B_LOC // 2:])
        nc.sync.dma_start(w1sb[:, 1:2], w1t[:, 1:2])
        nc.sync.dma_start(w2sb[:, 1:2], w2t[:, 1:2])
        for n, (c0, c1) in enumerate(W_CHUNKS[2:]):
            nc.sync.dma_start(w1sb[:, c0:c1], w1t[:, c0:c1])
            nc.sync.dma_start(w2sb[:, c0:c1], w2t[:, c0:c1])
            for p in range(2 * n, min(2 * n + 2, 8)):
                if p == 0:
                    # split pair 0 so class 2's half lands before its
                    # slot starts (the schedule is still filling the
                    # prefetch window here; later pairs have slack)
                    nc.sync.dma_start(xtps[0][:, :, 0], xtp[0, :, :, 0])
                    nc.sync.dma_start(xtps[0][:, :, 1], xtp[0, :, :, 1])
                else:
                    nc.sync.dma_start(xtps[p][:], xtp[p])
            if n == 0:
                nc.sync.dma_start(w3sb[:], w3z[:])
        nc.sync.dma_start(xts18[:], xt18[:])

        # ---- warm-up: keep the PE continuously busy (p-state ramps to
        # 2.4 GHz only after ~3 us of uninterrupted execution) with narrow
        # zero matmuls until the first X section + W1[0] land; the ps3
        # zero matmul also sets has_written for the shared GEMM3 bank.
        wu_l = consts.tile([128, 128], B16)
        wu_r = consts.tile([128, 512], B16)
        wu_sb = consts.tile([128, 2], F32)
        nc.vector.memset(wu_l[:], 0.0)
        wu_ps = ps_g.tile([128, 512], mybir.dt.float32, tag="pg")
        for i in range(4):
            nc.tensor.matmul(wu_ps[:, 0:128], wu_l[:], wu_l[:],
                             start=True, stop=True)
        nc.vector.memset(wu_r[:], 0.0)
        ps3 = ps_3.tile([128, SEC], mybir.dt.float32, tag="ps3")
        nc.tensor.matmul(ps3[:], wu_l[:], wu_r[:, 0:SEC],
                         start=True, stop=False, skip_group_check=True)
        for i in range(34):
            nc.tensor.matmul(wu_ps[:, 128:256], wu_l[:], wu_l[:],
                             start=True, stop=True)
        # preload the ACT engine's Relu table (one-time ~1.3 us) while the
        # first DMAs are still in flight, not on the critical G2(0) path.
        # Reads SBUF (wu_l), so it creates no hazard against the warmups.
        nc.scalar.activation(wu_sb[:, 0:2], wu_l[:, 0:2],
                             mybir.ActivationFunctionType.Relu)

        def filler(n):
            # dep-free narrow matmuls: keep the PE p-state ramp alive
            # through supply-driven stalls (idle resets the ramp; every
            # reset costs ~3 us of half-speed matmuls)
            for i in range(n):
                c0 = 128 + 64 * (i % 2)
                nc.tensor.matmul(wu_ps[:, c0:c0 + 64],
                                 wu_l[:], wu_l[:, 0:64],
                                 start=True, stop=True)

        w1v = w1sb[:]
        w2v = w2sb[:]
        w3v = w3sb[:]

        h1_t = [[None] * NSEC, [None] * NSEC]  # per class parity
        h2_t = [[None] * NSEC for _ in range(C)]

        def burst_h(r, h):
            # GEMM3 for classes 4r..4r+3, one section; strips concurrent.
            # Section h lands at partition offset 5h + r within strip j via
            # leading-zero columns in the lhsT (width 5h+r+1 <= 20); 5h+r
            # is a bijection onto 0..19 so the 20 partitions per strip are
            # dense and the out-DMA is contiguous.
            for k in range(2):
                for j in range(4):
                    c = 4 * r + j
                    if c >= C:
                        continue
                    w = 5 * h + r + 1
                    last = (r == 4 and h == NSEC - 1 and k == 1
                            and j == 2)
                    nc.tensor.matmul(
                        ps3[32 * j:32 * j + w, :],
                        w3v[:, c, k, h, 0:w],
                        h2_t[c][h][:, k, :],
                        start=False, stop=last,
                        tile_position=(0, 32 * j),
                        skip_group_check=True)

        def burst(r):
            for h in range(NSEC):
                burst_h(r, h)

        def g1_section(c, h):
            h1 = h1_pool.tile([128, 2, SEC], B16, tag="h1")
            for m in range(2):
                pg = ps_g.tile([128, SEC], mybir.dt.float32, tag="pg")
                for k in range(2):
                    nc.tensor.matmul(
                        pg[:], w1v[:, c, k, m, :],
                        x_ap(c, k, h * SEC, (h + 1) * SEC),
                        start=(k == 0), stop=(k == 1))
                nc.scalar.activation(
                    h1[:, m, :], pg[:],
                    mybir.ActivationFunctionType.Relu,
                    bias=b1sb[:, c, m:m+1])
            return h1

        def g2_section(c, h1, on_act):
            h2 = h2_pool.tile([128, 2, SEC], B16, tag="h2")
            for m in range(2):
                pg = ps_g.tile([128, SEC], mybir.dt.float32, tag="pg")
                for k in range(2):
                    nc.tensor.matmul(
                        pg[:], w2v[:, c, k, m, :], h1[:, k, :],
                        start=(k == 0), stop=(k == 1))
                if on_act:
                    # ACT evicts with no queue lag when GEMM1 is absent,
                    # so downstream deps resolve immediately.
                    nc.scalar.activation(
                        h2[:, m, :], pg[:],
                        mybir.ActivationFunctionType.Relu,
                        bias=b2sb[:, c, m:m+1])
                else:
                    nc.vector.tensor_scalar(
                        h2[:, m, :], pg[:],
                        b2sb[:, c, m:m+1], 0.0,
                        mybir.AluOpType.add, mybir.AluOpType.max)
            return h2

        # ---- class 0: section-level software pipeline (G1/G2 chained in
        # one slot). Halves the startup data-demand rate, so the PE never
        # outruns the DMA queue while it fills. Fillers between early
        # sections absorb residual supply stalls without resetting the
        # p-state ramp.
        h1s0 = [g1_section(0, 0)]
        filler(3)
        h1s0.append(g1_section(0, 1))
        filler(3)
        h2s0 = []
        for h in range(NSEC):
            if h + 2 < NSEC:
                h1s0.append(g1_section(0, h + 2))
                filler(2)
            h2s0.append(g2_section(0, h1s0[h], on_act=False))
            filler(2)
        h2_t[0] = h2s0

        for cc in range(1, C + 1):
            if cc >= 5 and (cc - 5) % 4 == 0 and cc < C:
                burst((cc - 5) // 4)
            # interleave G1(cc) and G2(cc-1) at section level: ACT (h1
            # eviction) and DVE (h2 eviction) load alternates evenly, so
            # the PE never outruns the 7 rotating PSUM banks, and the
            # last class's h2 is evicted mid-slot (burst leaders at the
            # next slot start no longer wait on it).
            for h in range(NSEC):
                if cc < C:
                    h1n = g1_section(cc, h)
                    if cc == 1 and h < 2:
                        filler(2)
                if cc >= 2:
                    h2_t[cc - 1][h] = g2_section(
                        cc - 1, h1_t[(cc - 1) % 2][h], on_act=(cc == C))
                if cc < C:
                    h1_t[cc % 2][h] = h1n
                if cc == C and h >= 1:
                    # final burst interleaved one section LAGGED, so each
                    # burst_h's h2(18, h-1) eviction has had a full
                    # section (~1.7 us) to drain; shortens the kernel tail
                    burst_h(4, h - 1)
            if cc == C:
                burst_h(4, NSEC - 1)

        # ---- eviction: single DVE PSUM->SBUF copy (ACT is still draining
        # the final class's h2 evictions; DVE is idle here), then one
        # 128-partition DMA (pad rows land in the dram padding).
        out_sb = out_pool.tile([128, SEC], F32, tag="osb")
        nc.vector.tensor_copy(out_sb[:], ps3[:])
        nc.sync.dma_start(out[:].rearrange("j q b -> (j q) b"), out_sb[:])

    _legalize_waits(nc)
    return nc


def _get_program():
    if 'nc' not in _CACHE:
        _setup_axon_env()
        _CACHE['nc'] = _build_program()
    return _CACHE['nc']


# ---------------------------------------------------------------------------
# host wrapper
# ---------------------------------------------------------------------------

def kernel(inputs, W1, b1, W2, b2, W3, b3):
    global last_results
    from concourse.bass_utils import run_bass_kernel_spmd

    nc = _get_program()

    inputs = np.asarray(inputs)
    W1 = np.asarray(W1, dtype=np.float32)
    b1 = np.asarray(b1, dtype=np.float32)
    W2 = np.asarray(W2, dtype=np.float32)
    b2 = np.asarray(b2, dtype=np.float32)
    W3 = np.asarray(W3, dtype=np.float32)
    b3 = np.asarray(b3, dtype=np.float32)

    # host-side layout prep: xtc[c, p, k, b] = x[b, 128k+p, c]
    xbf = inputs.reshape(B, 2, 128, C).astype(BF16)
    xtc_full = xbf.transpose(3, 2, 1, 0)      # (C, 128, 2, B)

    # lhsT tiles: w{1,2}t[p, c, k, m, j] = W[c, 128k+p, 128m+j]
    w1t = np.ascontiguousarray(
        W1.reshape(C, 2, 128, 2, 128).transpose(2, 0, 1, 3, 4)).astype(BF16)
    w2t = np.ascontiguousarray(
        W2.reshape(C, 2, 128, 2, 128).transpose(2, 0, 1, 3, 4)).astype(BF16)
    # w3z[p, c, k, h, i] = (i == 5h + c//4) * W3[c, 128k+p]
    w3z = np.zeros((128, C, 2, NSEC, 20), dtype=np.float32)
    for c in range(C):
        for h in range(NSEC):
            w3z[:, c, 0, h, 5 * h + c // 4] = W3[c, :128]
            w3z[:, c, 1, h, 5 * h + c // 4] = W3[c, 128:]
    w3z = w3z.astype(BF16)
    # b1s[p, c, m] = b1[c, 128m+p]
    b1s = np.ascontiguousarray(
        b1.reshape(C, 2, 128).transpose(2, 0, 1)).astype(np.float32)
    b2s = np.ascontiguousarray(
        b2.reshape(C, 2, 128).transpose(2, 0, 1)).astype(np.float32)

    core_ids = list(range(NCORES))
    in_maps = []
    for i in core_ids:
        xc = np.ascontiguousarray(
            xtc_full[:, :, :, i * B_LOC:(i + 1) * B_LOC])
        xtp = np.ascontiguousarray(
            xc[2:18].reshape(8, 2, 128, 2, B_LOC).transpose(0, 2, 3, 1, 4))
        in_maps.append({
            "xt01": xc[0:2], "xtp": xtp, "xt18": xc[18],
            "w1t": w1t, "w2t": w2t, "w3z": w3z, "b1s": b1s, "b2s": b2s,
        })

    import os
    trace = bool(os.environ.get("BASS_TRACE"))
    res = run_bass_kernel_spmd(nc, in_maps, core_ids, trace=trace)
    last_results = res

    out_full = np.empty((B, C), dtype=np.float32)
    for i in core_ids:
        o = res.results[i]["out"].reshape(4, 32, SEC)[:, :20]
        o = o.reshape(4, NSEC, 5, SEC)
        # o[j, h, r, b] = out[class 4r+j, batch 512h+b]
        o = o.transpose(1, 3, 2, 0).reshape(B_LOC, 20)[:, :C]
        out_full[i * B_LOC:(i + 1) * B_LOC] = o
    out_full += b3[None, :]
    return out_full

